# revision 6
# baseline (speedup 1.0000x reference)
"""Trainium2 Bass kernel for ComplexMultiheadAttention.

Sharding: core c = b*4 + g  (b = batch 0..1, g = head-group 0..3, 4 heads each).
All complex arithmetic is folded into stacked real matmuls via host-side weight
packing; on-device everything runs as fp32r (full-speed reduced-precision fp32)
K=128 matmuls with N=512 moving tiles.

Layouts (per core):
  xq/xk/xv : [128, 16, 2048]  packed stacked-transposed activations
             row e' = k*128+p of [x_r[b].T ; x_i[b].T]  (e' in 0..2047)
  wq/wk    : [128, 16, 512]   packed (complex-stacked projection weight).T
  wv       : [128, 16, 512]   packed stacked V weight (natural out layout)
  wo       : [128, 16, 512]   packed out-proj weight slice for this core
  qs/ks    : SBUF [128, 4, 2048]  per head h: rows 0:64 = q_r.T, 64:128 = q_i.T
  vs       : SBUF [128, 16, 512]  [l-chunk, j] with j = h*128 + (r/i)*64 + d
  scores   : S.T layout [key m (partitions), query l (free)] per head
  softmax  : exp without max subtraction (scores are O(+-20), safe in fp32);
             row sums via ones-matmul over partitions; normalizer broadcast
             via K=1 matmul; applied to PV output (deferred normalization)
  osT      : [128, 4, 2048] -> AllGather over the 4 cores of the batch ->
             [2048, 2048] stacked attention output; out-proj consumes it
  y        : [512, 2048] slice of [y_r.T ; y_i.T] (rows g*512..g*512+512)
"""

import os
import sys

for _p in ("/opt/trn_rl_repo",):
    if os.path.isdir(_p) and _p not in sys.path:
        sys.path.insert(0, _p)

import numpy as np

import concourse.bacc as bacc
import concourse.mybir as mybir
import concourse.tile as tile
from concourse import bass_utils

B, L, E, H = 2, 2048, 1024, 16
D = E // H          # 64
NCORES = 8
GROUPS = 4          # head-groups (tensor parallel inside a batch)
HL = H // GROUPS    # heads per core = 4
EL = HL * 2 * D     # stacked rows per core = 512
KC = 16             # 2048 / 128 contraction chunks
NT = L // 512       # 4 moving tiles over L
MT = EL // 128      # 4 output row tiles

F32 = mybir.dt.float32
F32R = mybir.dt.float32r
EXP = mybir.ActivationFunctionType.Exp
IDENT = mybir.ActivationFunctionType.Identity
MULT = mybir.AluOpType.mult


def build_nc(repeat: int = 1):
    nc = bacc.Bacc("TRN2", target_bir_lowering=False, debug=False,
                   num_devices=NCORES)

    xq = nc.dram_tensor("xq", [128, KC, L], F32R, kind="ExternalInput").ap()
    xk = nc.dram_tensor("xk", [128, KC, L], F32R, kind="ExternalInput").ap()
    xv = nc.dram_tensor("xv", [128, KC, L], F32R, kind="ExternalInput").ap()
    wq = nc.dram_tensor("wq", [128, KC, EL], F32R, kind="ExternalInput").ap()
    wk = nc.dram_tensor("wk", [128, KC, EL], F32R, kind="ExternalInput").ap()
    wv = nc.dram_tensor("wv", [128, KC, EL], F32R, kind="ExternalInput").ap()
    wo = nc.dram_tensor("wo", [128, KC, EL], F32R, kind="ExternalInput").ap()
    ones = nc.dram_tensor("ones", [128, 128], F32R, kind="ExternalInput").ap()
    bq = nc.dram_tensor("bq", [128, MT], F32, kind="ExternalInput").ap()
    bk = nc.dram_tensor("bk", [128, MT], F32, kind="ExternalInput").ap()
    bo = nc.dram_tensor("bo", [128, MT], F32, kind="ExternalInput").ap()
    bv = nc.dram_tensor("bv", [128, EL], F32, kind="ExternalInput").ap()
    y = nc.dram_tensor("y", [EL, L], F32, kind="ExternalOutput").ap()

    rg = [[0, 1, 2, 3], [4, 5, 6, 7]]

    with tile.TileContext(nc) as tc:
        with tc.tile_pool(name="persist", bufs=1) as persist:
            ones_t = persist.tile([128, 128], F32R)
            nc.sync.dma_start(ones_t[:], ones[:])
            bq_t = persist.tile([128, MT], F32)
            nc.sync.dma_start(bq_t[:], bq[:])
            bk_t = persist.tile([128, MT], F32)
            nc.sync.dma_start(bk_t[:], bk[:])
            bo_t = persist.tile([128, MT], F32)
            nc.sync.dma_start(bo_t[:], bo[:])
            bv_t = persist.tile([128, EL], F32)
            nc.sync.dma_start(bv_t[:], bv[:])

            for rep in range(repeat):
                _emit_body(nc, tc, rep, xq, xk, xv, wq, wk, wv, wo, y,
                           ones_t, bq_t, bk_t, bo_t, bv_t, rg)

    nc.compile()
    return nc


def _emit_body(nc, tc, rep, xq, xk, xv, wq, wk, wv, wo, y,
               ones_t, bq_t, bk_t, bo_t, bv_t, rg):
    ag_in = nc.dram_tensor(f"ag_in_{rep}", [EL, L], F32R).ap()
    ag_out = nc.dram_tensor(f"ag_out_{rep}", [GROUPS * EL, L], F32R).ap()

    with tc.tile_pool(name="qkv_sb", bufs=1) as qkv_sb:
        qs_sb = qkv_sb.tile([128, HL, L], F32R)
        ks_sb = qkv_sb.tile([128, HL, L], F32R)
        vs_sb = qkv_sb.tile([128, KC, EL], F32R)
        os_sb = qkv_sb.tile([128, HL, L], F32R)

        # ---------------- Q / K projections ----------------
        for x_d, w_d, out_sb, bias_t in ((xq, wq, qs_sb, bq_t),
                                         (xk, wk, ks_sb, bk_t)):
            with tc.tile_pool(name="wp", bufs=1) as wp, \
                 tc.tile_pool(name="xp", bufs=4) as xp, \
                 tc.tile_pool(name="pp", bufs=6, space="PSUM") as pp:
                w_t = wp.tile([128, KC, EL], F32R)
                nc.sync.dma_start(w_t[:], w_d[:])
                for n in range(NT):
                    ls = slice(n * 512, (n + 1) * 512)
                    accs = [pp.tile([128, 512], F32, name=f"qk_acc{m}",
                                    tag="qk_acc")
                            for m in range(MT)]
                    for k in range(KC):
                        xt = xp.tile([128, 512], F32R, name="xqk")
                        nc.sync.dma_start(xt[:], x_d[:, k, ls])
                        for m in range(MT):
                            nc.tensor.matmul(
                                accs[m][:],
                                w_t[:, k, m * 128:(m + 1) * 128],
                                xt[:],
                                start=(k == 0), stop=(k == KC - 1))
                    for m in range(MT):
                        nc.scalar.activation(out_sb[:, m, ls], accs[m][:],
                                             IDENT, bias=bias_t[:, m:m + 1])

        # ---------------- V projection ----------------
        with tc.tile_pool(name="wp", bufs=1) as wp, \
             tc.tile_pool(name="xp", bufs=6) as xp, \
             tc.tile_pool(name="pp", bufs=3, space="PSUM") as pp:
            w_t = wp.tile([128, KC, EL], F32R)
            nc.sync.dma_start(w_t[:], wv[:])
            for lc in range(KC):
                acc = pp.tile([128, EL], F32, name="v_acc")
                for k in range(KC):
                    xt = xp.tile([128, 128], F32R, name="xv_t")
                    nc.sync.dma_start(xt[:], xv[:, k, lc * 128:(lc + 1) * 128])
                    nc.tensor.matmul(acc[:], xt[:], w_t[:, k, :],
                                     start=(k == 0), stop=(k == KC - 1))
                nc.vector.tensor_add(vs_sb[:, lc, :], acc[:], bv_t[:])

        # ---------------- attention (per head) ----------------
        with tc.tile_pool(name="scp", bufs=3, space="PSUM") as scp, \
             tc.tile_pool(name="pvp", bufs=4, space="PSUM") as pvp, \
             tc.tile_pool(name="ep", bufs=3) as ep, \
             tc.tile_pool(name="accp", bufs=2) as accp, \
             tc.tile_pool(name="tiny", bufs=2) as tiny:
            for h in range(HL):
                pv = [pvp.tile([128, 512], F32, name=f"pv{n}", tag="pv")
                      for n in range(NT)]
                acc0 = accp.tile([128, L], F32R, name="acc0")
                acc1 = accp.tile([128, L], F32R, name="acc1")
                for mc in range(KC):
                    ms = slice(mc * 128, (mc + 1) * 128)
                    if mc == 0:
                        ex = acc0
                    elif mc == 1:
                        ex = acc1
                    else:
                        ex = ep.tile([128, L], F32R, name="ex")
                    for n in range(NT):
                        ls = slice(n * 512, (n + 1) * 512)
                        sc = scp.tile([128, 512], F32, name="sc", tag="sc")
                        nc.tensor.matmul(sc[:], ks_sb[:, h, ms],
                                         qs_sb[:, h, ls],
                                         start=True, stop=True)
                        nc.scalar.activation(ex[:, ls], sc[:], EXP,
                                             scale=float(1.0 / np.sqrt(D)))
                        nc.tensor.matmul(pv[n][:],
                                         vs_sb[:, mc, h * 128:(h + 1) * 128],
                                         ex[:, ls],
                                         start=(mc == 0), stop=(mc == KC - 1))
                    if mc >= 2:
                        tgt = acc0 if mc % 2 == 0 else acc1
                        nc.vector.tensor_add(tgt[:], tgt[:], ex[:])
                nc.vector.tensor_add(acc0[:], acc0[:], acc1[:])
                # normalize: r = 1/colsum ; os = pv * bcast(r)
                for n in range(NT):
                    ls = slice(n * 512, (n + 1) * 512)
                    rs = scp.tile([128, 512], F32, name="rs", tag="sc")
                    nc.tensor.matmul(rs[:1, :], ones_t[:, :1], acc0[:, ls],
                                     start=True, stop=True)
                    r_sb = tiny.tile([1, 512], F32R, name="r_sb")
                    with nc.allow_low_precision(reason="fp32r stores fp32"):
                        nc.vector.reciprocal(r_sb[:], rs[:1, :])
                    bc = scp.tile([128, 512], F32, name="bc", tag="sc")
                    nc.tensor.matmul(bc[:], ones_t[:1, :], r_sb[:],
                                     start=True, stop=True)
                    bc_sb = ep.tile([128, 512], F32, name="bc_sb")
                    nc.vector.tensor_copy(bc_sb[:], bc[:])
                    nc.vector.tensor_tensor(os_sb[:, h, ls], pv[n][:],
                                            bc_sb[:], MULT)

        # ---------------- AllGather ----------------
        ag_in_v = ag_in.rearrange("(h p) l -> h p l", p=128)
        for h in range(HL):
            nc.sync.dma_start(ag_in_v[h], os_sb[:, h, :])
        nc.gpsimd.collective_compute(
            "AllGather", mybir.AluOpType.bypass, replica_groups=rg,
            ins=[ag_in.opt()], outs=[ag_out.opt()])

        # ---------------- out projection ----------------
        with tc.tile_pool(name="wp", bufs=1) as wp, \
             tc.tile_pool(name="ogp", bufs=4) as ogp, \
             tc.tile_pool(name="pp", bufs=6, space="PSUM") as pp, \
             tc.tile_pool(name="yp", bufs=3) as yp:
            w_t = wp.tile([128, KC, EL], F32R)
            nc.sync.dma_start(w_t[:], wo[:])
            for n in range(NT):
                ls = slice(n * 512, (n + 1) * 512)
                accs = [pp.tile([128, 512], F32, name=f"o_acc{m}",
                                tag="o_acc")
                        for m in range(MT)]
                for k in range(KC):
                    og = ogp.tile([128, 512], F32R, name="og")
                    nc.sync.dma_start(og[:], ag_out[k * 128:(k + 1) * 128, ls])
                    for m in range(MT):
                        nc.tensor.matmul(
                            accs[m][:], w_t[:, k, m * 128:(m + 1) * 128],
                            og[:], start=(k == 0), stop=(k == KC - 1))
                for m in range(MT):
                    yt = yp.tile([128, 512], F32, name="yt")
                    nc.scalar.activation(yt[:], accs[m][:], IDENT,
                                         bias=bo_t[:, m:m + 1])
                    nc.sync.dma_start(y[m * 128:(m + 1) * 128, ls], yt[:])


def _pack(a, rows=128):
    """[rows*KC', F] -> [rows, KC', F] with row k*rows+p -> [p, k]."""
    kc = a.shape[0] // rows
    return np.ascontiguousarray(
        a.reshape(kc, rows, *a.shape[1:]).transpose(1, 0, 2))


def _stack_qk_w(Wr, Wi, g):
    """Transposed stacked projection weight [2048, 512] for head-group g."""
    hsl = slice(g * HL * D, (g + 1) * HL * D)
    top = np.concatenate([Wr[hsl].T, -Wi[hsl].T], axis=0)  # part=0 cols
    bot = np.concatenate([Wi[hsl].T, Wr[hsl].T], axis=0)   # part=1 cols
    return np.ascontiguousarray(
        np.stack([top.reshape(2 * E, HL, D), bot.reshape(2 * E, HL, D)],
                 axis=2).reshape(2 * E, EL))


def _stack_v_w(Wr, Wi, g):
    """Stacked V weight [2048, 512] (natural-out layout) for head-group g."""
    hsl = slice(g * HL * D, (g + 1) * HL * D)
    p0 = np.concatenate([Wr[hsl].T, -Wi[hsl].T], axis=0)
    p1 = np.concatenate([Wi[hsl].T, Wr[hsl].T], axis=0)
    return np.ascontiguousarray(
        np.stack([p0.reshape(2 * E, HL, D), p1.reshape(2 * E, HL, D)],
                 axis=2).reshape(2 * E, EL))


def _stack_bias(br, bi, g):
    hsl = slice(g * HL * D, (g + 1) * HL * D)
    s = np.stack([br[hsl].reshape(HL, D), bi[hsl].reshape(HL, D)],
                 axis=1).reshape(EL)
    return np.ascontiguousarray(s.reshape(MT, 128).T)  # [128, MT]


def prep_in_maps(inputs):
    f32 = np.float32
    xs = {}
    for b in range(B):
        for nm, xr, xi in (("xq", inputs["query_r"], inputs["query_i"]),
                           ("xk", inputs["key_r"], inputs["key_i"]),
                           ("xv", inputs["value_r"], inputs["value_i"])):
            stk = np.concatenate([np.asarray(xr[b]).T, np.asarray(xi[b]).T],
                                 axis=0).astype(f32)     # [2048, L]
            xs[(nm, b)] = _pack(stk)

    # out-proj: full stacked weight [e''=2048, out_row=2048]
    WoT_r = np.asarray(inputs["Wo_r"]).T.astype(f32)
    WoT_i = np.asarray(inputs["Wo_i"]).T.astype(f32)
    top = np.concatenate([WoT_r, WoT_i], axis=1)    # part=0 rows
    bot = np.concatenate([-WoT_i, WoT_r], axis=1)   # part=1 rows
    inter = np.stack([top.reshape(H, D, 2 * E), bot.reshape(H, D, 2 * E)],
                     axis=1).reshape(2 * E, 2 * E)  # [(head,part,d), row]
    bo_cat = np.concatenate([np.asarray(inputs["bo_r"]),
                             np.asarray(inputs["bo_i"])]).astype(f32)

    ones = np.ones((128, 128), dtype=f32)
    in_maps = []
    for c in range(NCORES):
        b, g = divmod(c, GROUPS)
        hsl = slice(g * HL * D, (g + 1) * HL * D)
        bv_s = np.stack([np.asarray(inputs["bv_r"])[hsl].reshape(HL, D),
                         np.asarray(inputs["bv_i"])[hsl].reshape(HL, D)],
                        axis=1).reshape(EL).astype(f32)
        m = {
            "xq": xs[("xq", b)], "xk": xs[("xk", b)], "xv": xs[("xv", b)],
            "wq": _pack(_stack_qk_w(np.asarray(inputs["Wq_r"], f32),
                                    np.asarray(inputs["Wq_i"], f32), g)),
            "wk": _pack(_stack_qk_w(np.asarray(inputs["Wk_r"], f32),
                                    np.asarray(inputs["Wk_i"], f32), g)),
            "wv": _pack(_stack_v_w(np.asarray(inputs["Wv_r"], f32),
                                   np.asarray(inputs["Wv_i"], f32), g)),
            "wo": _pack(np.ascontiguousarray(
                inter[:, g * EL:(g + 1) * EL])),
            "ones": ones,
            "bq": _stack_bias(np.asarray(inputs["bq_r"], f32),
                              np.asarray(inputs["bq_i"], f32), g),
            "bk": _stack_bias(np.asarray(inputs["bk_r"], f32),
                              np.asarray(inputs["bk_i"], f32), g),
            "bo": np.ascontiguousarray(
                bo_cat[g * EL:(g + 1) * EL].reshape(MT, 128).T),
            "bv": np.broadcast_to(bv_s, (128, EL)).copy(),
        }
        in_maps.append(m)
    return in_maps


def assemble(results):
    out = np.empty((2, B, L, E), np.float32)
    for b in range(B):
        ys = np.concatenate([results[b * GROUPS + g]["y"]
                             for g in range(GROUPS)], axis=0)  # [2048, L]
        out[0, b] = ys[:E].T
        out[1, b] = ys[E:].T
    return out


_NC_CACHE = {}


def get_nc(repeat: int = 1):
    if repeat not in _NC_CACHE:
        _NC_CACHE[repeat] = build_nc(repeat)
    return _NC_CACHE[repeat]


def make_runner(nc):
    """Build a reusable jitted SPMD executor for `nc` (compiles once).

    Mirrors concourse.bass2jax.run_bass_via_pjrt's multi-core path, but the
    jitted callable is constructed a single time so repeated invocations do
    not re-trigger the walrus/NEFF compile.
    """
    import jax
    from jax.experimental.shard_map import shard_map
    from jax.sharding import Mesh, PartitionSpec

    from concourse import bass2jax

    bass2jax.install_neuronx_cc_hook()
    assert nc.dbg_addr is None

    partition_name = (nc.partition_id_tensor.name
                      if nc.partition_id_tensor else None)
    in_names, out_names, out_avals, zero_outs = [], [], [], []
    for alloc in nc.m.functions[0].allocations:
        if not isinstance(alloc, mybir.MemoryLocationSet):
            continue
        name = alloc.memorylocations[0].name
        if alloc.kind == "ExternalInput":
            if name != partition_name:
                in_names.append(name)
        elif alloc.kind == "ExternalOutput":
            shape = tuple(alloc.tensor_shape)
            dtype = mybir.dt.np(alloc.dtype)
            out_names.append(name)
            out_avals.append(jax.core.ShapedArray(shape, dtype))
            zero_outs.append(np.zeros(shape, dtype))
    n_params = len(in_names)
    n_outs = len(out_avals)
    all_in_names = list(in_names) + list(out_names)
    if partition_name is not None:
        all_in_names.append(partition_name)

    def _body(*args):
        operands = list(args)
        if partition_name is not None:
            operands.append(bass2jax.partition_id_tensor())
        outs = bass2jax._bass_exec_p.bind(
            *operands,
            out_avals=tuple(out_avals),
            in_names=tuple(all_in_names),
            out_names=tuple(out_names),
            lowering_input_output_aliases=(),
            sim_require_finite=True,
            sim_require_nnan=True,
            nc=nc,
        )
        return tuple(outs)

    devices = jax.devices()[:NCORES]
    mesh = Mesh(np.asarray(devices), ("core",))
    specs_in = (PartitionSpec("core"),) * (n_params + n_outs)
    specs_out = (PartitionSpec("core"),) * n_outs
    donate = tuple(range(n_params, n_params + n_outs))
    sharded = jax.jit(
        shard_map(_body, mesh=mesh, in_specs=specs_in, out_specs=specs_out,
                  check_rep=False),
        donate_argnums=donate, keep_unused=True)

    def run(in_maps, device_inputs=None):
        if device_inputs is None:
            device_inputs = put_inputs(in_maps)
        concat_zeros = [
            np.zeros((NCORES * z.shape[0], *z.shape[1:]), z.dtype)
            for z in zero_outs]
        out_arrs = sharded(*device_inputs, *concat_zeros)
        jax.block_until_ready(out_arrs)
        return [
            {name: np.asarray(out_arrs[i]).reshape(
                NCORES, *out_avals[i].shape)[c]
             for i, name in enumerate(out_names)}
            for c in range(NCORES)]

    def put_inputs(in_maps):
        return [
            np.concatenate([np.asarray(in_maps[c][nm])
                            for c in range(NCORES)], axis=0)
            for nm in in_names]

    def put_device(in_maps):
        from jax.sharding import NamedSharding
        sh = NamedSharding(mesh, PartitionSpec("core"))
        arrs = [jax.device_put(a, sh) for a in put_inputs(in_maps)]
        jax.block_until_ready(arrs)
        return arrs

    run.put_inputs = put_inputs
    run.put_device = put_device
    return run


_RUNNER_CACHE = {}


def get_runner(repeat: int = 1):
    if repeat not in _RUNNER_CACHE:
        _RUNNER_CACHE[repeat] = make_runner(get_nc(repeat))
    return _RUNNER_CACHE[repeat]


def kernel(**inputs) -> np.ndarray:
    runner = get_runner(1)
    in_maps = prep_in_maps(inputs)
    results = runner(in_maps)
    return assemble(results)


if __name__ == "__main__":
    pass
